# revision 43
# baseline (speedup 1.0000x reference)
"""Differentiable top-k (topk_masking) Trainium2 Bass kernel.

Problem: similarities [64, 131072] f32 ->
  hard_indices [64, 5] int32  (top-5 per row, descending, ties by lowest index)
  soft_weights [64, 5, 131072] f32

The reference's straight-through output `(onehot + p) - p` is exactly 0.0 at
non-selected positions and within 1 ulp of 1.0 at the selected position, so the
kernel emits an exact one-hot (the runtime pre-zeroes ExternalOutput buffers;
only the 5 ones per row are scattered).

Sharding: pure data parallel — 8 rows per core across 8 NeuronCores.

Per-core pipeline (local block [8, 131072] viewed as [128, 8192]; partition p
holds row p//16, in-row chunk p%16):
  1. Input is loaded in 4 column-chunks of [128, 2048]; per chunk, MAX8 +
     FIND_INDEX8 give the per-partition top-8 (values + flat indices), so the
     Vector engine pipelines behind the DMA.
  2. The 32 per-partition candidates are prefiltered back to top-5 (MAX8 over
     the 32, then a one-hot dot to gather their flat indices).
  3. Identity-order SBUF->SBUF DMAs regroup each row's 16x(5 vals + 5 idx)
     candidates into one partition; MAX8/FIND_INDEX8 rank the 80 candidates
     (duplicate-value index semantics match jax.lax.top_k tie order).
  4. The winning flat indices drive an indirect-DMA scatter of 1.0s into the
     dense output, plus the hard_indices store.
"""

import numpy as np

import concourse.bacc as bacc
import concourse.bass as bass
import concourse.mybir as mybir
import concourse.tile as tile
from concourse.bass import IndirectOffsetOnAxis
from concourse.bass_utils import run_bass_kernel_spmd

B = 64
D = 131072
K = 5
N_CORES = 8
B_LOC = B // N_CORES          # 8 rows per core
P = 128                       # SBUF partitions
FREE = B_LOC * D // P         # 8192 elems per partition
NCH = P // B_LOC              # 16 partitions (chunks) per row
NCHK = 4                      # input load chunks
CWS = [1024, 1536, 2304, 3328]  # small first chunks start Vector early
COFF = [0, 1024, 2560, 4864]    # per-partition column offsets
assert sum(CWS) == FREE and COFF[-1] + CWS[-1] == FREE
NCAND = NCH * K               # 80 candidates per row after prefilter (top-5
                              # per partition provably covers the row top-5)

f32 = mybir.dt.float32
i32 = mybir.dt.int32
u32 = mybir.dt.uint32
ADD = mybir.AluOpType.add
SUB = mybir.AluOpType.subtract
MUL = mybir.AluOpType.mult
EQ = mybir.AluOpType.is_equal
AXX = mybir.AxisListType.X

_NC_CACHE = None


def _mid_bcast(ap, n):
    """[p, m] -> [p, n, m] with a step-0 middle dim."""
    dims = [list(d) for d in ap.ap]
    return bass.AP(ap.tensor, ap.offset, [dims[0], [0, n]] + dims[1:])


def build_nc():
    nc = bacc.Bacc("TRN2", target_bir_lowering=False, debug=False)
    # chunk-major input layout: chunk c is a contiguous [128, CWS[c]] block
    x_dram = nc.dram_tensor("sims", [P * FREE], f32, kind="ExternalInput")
    soft_dram = nc.dram_tensor("soft", [B_LOC * K * D], f32, kind="ExternalOutput")
    hard_dram = nc.dram_tensor("hard", [B_LOC, K], i32, kind="ExternalOutput")

    with tile.TileContext(nc) as tc:
        with tc.tile_pool(name="sb", bufs=1) as pool:
            # ---- constants, no deps (gpsimd; overlap the input DMA) ----
            pbase = pool.tile([P, 1], f32)  # p * FREE
            nc.gpsimd.iota(
                pbase[:], pattern=[[0, 1]], base=0, channel_multiplier=FREE,
                allow_small_or_imprecise_dtypes=True,
            )
            iota832 = pool.tile([P, K * 8 * NCHK], f32)  # c in 0..31, repeated 5x
            nc.gpsimd.iota(
                iota832[:], pattern=[[0, K], [1, 8 * NCHK]], base=0,
                channel_multiplier=0, allow_small_or_imprecise_dtypes=True,
            )
            iota5128 = pool.tile([B_LOC, K * NCAND], f32)  # c in 0..127, rep 5x
            nc.gpsimd.iota(
                iota5128[:], pattern=[[0, K], [1, NCAND]], base=0,
                channel_multiplier=0, allow_small_or_imprecise_dtypes=True,
            )
            rowbase = pool.tile([B_LOC, 1], f32)  # r * D
            nc.gpsimd.iota(
                rowbase[:], pattern=[[0, 1]], base=0, channel_multiplier=1,
                allow_small_or_imprecise_dtypes=True,
            )
            nc.gpsimd.tensor_scalar_mul(rowbase[:], rowbase[:], float(D))
            obase = pool.tile([B_LOC, K], f32)  # (4r + j) * D
            nc.gpsimd.iota(
                obase[:], pattern=[[1, K]], base=0, channel_multiplier=K - 1,
                allow_small_or_imprecise_dtypes=True,
            )
            nc.gpsimd.tensor_scalar_mul(obase[:], obase[:], float(D))
            ones40 = pool.tile([B_LOC * K, 1], f32)
            nc.gpsimd.memset(ones40[:], 1.0)

            # ---- stage 1: chunked load + per-partition top-8 ----
            # all chunks on ONE ring: serial full-bandwidth transfers give the
            # earliest chunk0 (two rings split HBM bandwidth and delay it)
            xs = []
            for c in range(NCHK):
                xc = pool.tile([P, CWS[c]], f32, tag=f"x{c}")
                seg = x_dram[P * COFF[c] : P * (COFF[c] + CWS[c])]
                nc.sync.dma_start(
                    out=xc[:], in_=seg.rearrange("(p w) -> p w", w=CWS[c])
                )
                xs.append(xc)

            v32 = pool.tile([P, 8 * NCHK], f32)   # candidate values
            i32u = pool.tile([P, 8 * NCHK], u32)
            i32f = pool.tile([P, 8 * NCHK], f32)
            g32 = pool.tile([P, 8 * NCHK], f32)   # candidate flat indices
            for c in range(NCHK):
                sl = slice(8 * c, 8 * c + 8)
                nc.vector.max(out=v32[:, sl], in_=xs[c][:])
                nc.vector.max_index(
                    out=i32u[:, sl], in_max=v32[:, sl], in_values=xs[c][:]
                )
                # last chunk's index ops on Vector: no cross-engine wait on the
                # critical path into stage 2
                eng = nc.vector if c == NCHK - 1 else nc.gpsimd
                eng.tensor_copy(out=i32f[:, sl], in_=i32u[:, sl])
                eng.tensor_scalar(
                    out=g32[:, sl], in0=i32f[:, sl], scalar1=pbase[:, 0:1],
                    scalar2=float(COFF[c]), op0=ADD, op1=ADD,
                )

            # ---- stage 2: prefilter 32 -> 8 per partition ----
            vg2v = pool.tile([P, 8], f32)   # prefiltered values (MAX8 emits 8)
            vg2i = pool.tile([P, K], f32)   # flat indices of the top-5 kept
            nc.vector.max(out=vg2v[:], in_=v32[:])
            c8p = pool.tile([P, 8], u32)
            nc.vector.max_index(out=c8p[:], in_max=vg2v[:], in_values=v32[:])
            # values half of the regroup can fly while the index half computes:
            # flat order of [128,0:5] (p-major) == [8,80] (r-major) -> one DMA
            v128 = pool.tile([B_LOC, NCAND], f32)
            nc.sync.dma_start(out=v128[:], in_=vg2v[:, 0:K])
            c8pf = pool.tile([P, K], f32)
            nc.vector.tensor_copy(out=c8pf[:], in_=c8p[:, 0:K])
            mp = pool.tile([P, K * 8 * NCHK], f32)
            mp3 = mp[:].rearrange("p (j c) -> p j c", j=K)
            nc.vector.tensor_tensor(
                out=mp3, in0=iota832[:].rearrange("p (j c) -> p j c", j=K),
                in1=c8pf[:].to_broadcast([P, K, 8 * NCHK]), op=EQ,
            )
            nc.vector.tensor_tensor(
                out=mp3, in0=mp3, in1=_mid_bcast(g32[:], K), op=MUL
            )
            nc.vector.tensor_reduce(out=vg2i[:], in_=mp3, axis=AXX, op=ADD)
            idx2 = pool.tile([B_LOC, NCAND], f32)
            nc.sync.dma_start(out=idx2[:], in_=vg2i[:])

            # ---- stage 3: rank the 80 candidates per row ----
            t8 = pool.tile([B_LOC, 8], f32)
            nc.vector.max(out=t8[:], in_=v128[:])
            c8 = pool.tile([B_LOC, 8], u32)
            nc.vector.max_index(out=c8[:], in_max=t8[:], in_values=v128[:])
            c8f = pool.tile([B_LOC, 8], f32)
            nc.vector.tensor_copy(out=c8f[:], in_=c8[:])

            # ---- stage 4: gather the 5 winners' flat indices ----
            m5 = pool.tile([B_LOC, K * NCAND], f32)
            m53 = m5[:].rearrange("p (j c) -> p j c", j=K)
            nc.vector.tensor_tensor(
                out=m53, in0=iota5128[:].rearrange("p (j c) -> p j c", j=K),
                in1=c8f[:, 0:K].to_broadcast([B_LOC, K, NCAND]), op=EQ,
            )
            nc.vector.tensor_tensor(
                out=m53, in0=m53, in1=_mid_bcast(idx2[:], K), op=MUL
            )
            gsel = pool.tile([B_LOC, K], f32)
            nc.vector.tensor_reduce(out=gsel[:], in_=m53, axis=AXX, op=ADD)

            # ---- outputs ----
            # hard_indices = flat - r*D  (gpsimd, parallel with the off chain)
            hard_f = pool.tile([B_LOC, K], f32)
            nc.gpsimd.tensor_scalar(
                out=hard_f[:], in0=gsel[:], scalar1=rowbase[:, 0:1], scalar2=None,
                op0=SUB,
            )
            hard_t = pool.tile([B_LOC, K], i32)
            nc.gpsimd.tensor_copy(out=hard_t[:], in_=hard_f[:])
            nc.scalar.dma_start(out=hard_dram[:], in_=hard_t[:])

            # scatter offsets: flat + (4r + j)*D = (5r + j)*D + within
            off_f = pool.tile([B_LOC, K], f32)
            nc.vector.tensor_tensor(out=off_f[:], in0=gsel[:], in1=obase[:], op=ADD)
            off = pool.tile([B_LOC, K], i32)
            nc.vector.tensor_copy(out=off[:], in_=off_f[:])
            # regroup [8,5] -> [40,1]: flat element order matches, one DMA
            # (the HW DGE consumes one offset per partition)
            off40 = pool.tile([B_LOC * K, 1], i32)
            nc.sync.dma_start(out=off40[:], in_=off[:])
            nc.gpsimd.indirect_dma_start(
                out=soft_dram[:].rearrange("(a b) -> a b", b=1),
                out_offset=IndirectOffsetOnAxis(ap=off40[:], axis=0),
                in_=ones40[:],
                in_offset=None,
            )

    nc.compile()
    return nc


def get_nc():
    global _NC_CACHE
    if _NC_CACHE is None:
        _NC_CACHE = build_nc()
    return _NC_CACHE


def _kernel_subprocess(similarities: np.ndarray) -> tuple[np.ndarray, np.ndarray]:
    import os
    import subprocess
    import sys
    import tempfile

    here = os.path.dirname(os.path.abspath(__file__))
    with tempfile.TemporaryDirectory() as td:
        np.save(os.path.join(td, "in.npy"), similarities)
        code = (
            "import sys, numpy as np\n"
            f"sys.path.insert(0, {here!r})\n"
            "import kernel as KM\n"
            f"sims = np.load({os.path.join(td, 'in.npy')!r})\n"
            "hard, soft = KM.kernel(sims, _no_fallback=True)\n"
            f"np.save({os.path.join(td, 'hard.npy')!r}, hard)\n"
            f"np.save({os.path.join(td, 'soft.npy')!r}, soft)\n"
        )
        subprocess.run([sys.executable, "-c", code], check=True, cwd=td)
        return (
            np.load(os.path.join(td, "hard.npy")),
            np.load(os.path.join(td, "soft.npy")),
        )


def kernel(
    similarities: np.ndarray,
    _trace: bool = False,
    _tmpdir: str | None = None,
    _no_fallback: bool = False,
) -> tuple[np.ndarray, np.ndarray]:
    similarities = np.ascontiguousarray(np.asarray(similarities, dtype=np.float32))
    assert similarities.shape == (B, D)

    nc = get_nc()

    def chunked(block):
        b2 = block.reshape(P, FREE)
        return np.concatenate(
            [b2[:, COFF[c] : COFF[c] + CWS[c]].ravel() for c in range(NCHK)]
        )

    in_maps = [
        {"sims": chunked(similarities[c * B_LOC : (c + 1) * B_LOC])}
        for c in range(N_CORES)
    ]
    try:
        res = run_bass_kernel_spmd(
            nc, in_maps, core_ids=list(range(N_CORES)), trace=_trace, tmpdir=_tmpdir
        )
    except Exception:
        if _no_fallback:
            raise
        # rare transient device faults: isolate a retry in a fresh process
        return _kernel_subprocess(similarities)
    results = res.results if hasattr(res, "results") else res
    if _trace:
        kernel.last_results = res

    hard = np.concatenate(
        [np.asarray(r["hard"], dtype=np.int32) for r in results], axis=0
    )
    soft = np.concatenate(
        [
            np.asarray(r["soft"], dtype=np.float32).reshape(B_LOC, K, D)
            for r in results
        ],
        axis=0,
    )
    return hard, soft


# revision 46
# speedup vs baseline: 1.0138x; 1.0138x over previous
"""Differentiable top-k (topk_masking) Trainium2 Bass kernel.

Problem: similarities [64, 131072] f32 ->
  hard_indices [64, 5] int32  (top-5 per row, descending, ties by lowest index)
  soft_weights [64, 5, 131072] f32

The reference's straight-through output `(onehot + p) - p` is exactly 0.0 at
non-selected positions and within 1 ulp of 1.0 at the selected position, so the
kernel emits an exact one-hot (the runtime pre-zeroes ExternalOutput buffers;
only the 5 ones per row are scattered).

Sharding: pure data parallel — 8 rows per core across 8 NeuronCores.

Per-core pipeline (local block [8, 131072] viewed as [128, 8192]; partition p
holds row p//16, in-row chunk p%16):
  1. Input is loaded in 4 column-chunks of [128, 2048]; per chunk, MAX8 +
     FIND_INDEX8 give the per-partition top-8 (values + flat indices), so the
     Vector engine pipelines behind the DMA.
  2. The 32 per-partition candidates are prefiltered back to top-5 (MAX8 over
     the 32, then a one-hot dot to gather their flat indices).
  3. Identity-order SBUF->SBUF DMAs regroup each row's 16x(5 vals + 5 idx)
     candidates into one partition; MAX8/FIND_INDEX8 rank the 80 candidates
     (duplicate-value index semantics match jax.lax.top_k tie order).
  4. The winning flat indices drive an indirect-DMA scatter of 1.0s into the
     dense output, plus the hard_indices store.
"""

import numpy as np

import concourse.bacc as bacc
import concourse.bass as bass
import concourse.mybir as mybir
import concourse.tile as tile
from concourse.bass import IndirectOffsetOnAxis
from concourse.bass_utils import run_bass_kernel_spmd

B = 64
D = 131072
K = 5
N_CORES = 8
B_LOC = B // N_CORES          # 8 rows per core
P = 128                       # SBUF partitions
FREE = B_LOC * D // P         # 8192 elems per partition
NCH = P // B_LOC              # 16 partitions (chunks) per row
NCHK = 4                      # input load chunks
CWS = [1024, 1536, 2304, 3328]  # small first chunks start Vector early
COFF = [0, 1024, 2560, 4864]    # per-partition column offsets
assert sum(CWS) == FREE and COFF[-1] + CWS[-1] == FREE
NCAND = NCH * K               # 80 candidates per row after prefilter (top-5
                              # per partition provably covers the row top-5)

f32 = mybir.dt.float32
i32 = mybir.dt.int32
u32 = mybir.dt.uint32
ADD = mybir.AluOpType.add
SUB = mybir.AluOpType.subtract
MUL = mybir.AluOpType.mult
EQ = mybir.AluOpType.is_equal
AXX = mybir.AxisListType.X

_NC_CACHE = None


def _mid_bcast(ap, n):
    """[p, m] -> [p, n, m] with a step-0 middle dim."""
    dims = [list(d) for d in ap.ap]
    return bass.AP(ap.tensor, ap.offset, [dims[0], [0, n]] + dims[1:])


def build_nc():
    nc = bacc.Bacc("TRN2", target_bir_lowering=False, debug=False)
    # chunk-major input layout: chunk c is a contiguous [128, CWS[c]] block
    x_dram = nc.dram_tensor("sims", [P * FREE], f32, kind="ExternalInput")
    soft_dram = nc.dram_tensor("soft", [B_LOC * K * D], f32, kind="ExternalOutput")
    hard_dram = nc.dram_tensor("hard", [B_LOC, K], i32, kind="ExternalOutput")

    with tile.TileContext(nc) as tc:
        with tc.tile_pool(name="sb", bufs=1) as pool:
            # ---- constants, no deps (gpsimd; overlap the input DMA) ----
            pbase = pool.tile([P, 1], f32)  # p * FREE
            nc.gpsimd.iota(
                pbase[:], pattern=[[0, 1]], base=0, channel_multiplier=FREE,
                allow_small_or_imprecise_dtypes=True,
            )
            iota832 = pool.tile([P, K * 8 * NCHK], f32)  # c in 0..31, repeated 5x
            nc.gpsimd.iota(
                iota832[:], pattern=[[0, K], [1, 8 * NCHK]], base=0,
                channel_multiplier=0, allow_small_or_imprecise_dtypes=True,
            )
            iota5128 = pool.tile([B_LOC, K * NCAND], f32)  # c in 0..127, rep 5x
            nc.gpsimd.iota(
                iota5128[:], pattern=[[0, K], [1, NCAND]], base=0,
                channel_multiplier=0, allow_small_or_imprecise_dtypes=True,
            )
            rowbase = pool.tile([B_LOC, 1], f32)  # r * D
            nc.gpsimd.iota(
                rowbase[:], pattern=[[0, 1]], base=0, channel_multiplier=1,
                allow_small_or_imprecise_dtypes=True,
            )
            nc.gpsimd.tensor_scalar_mul(rowbase[:], rowbase[:], float(D))
            obase = pool.tile([B_LOC, K], f32)  # (4r + j) * D
            nc.gpsimd.iota(
                obase[:], pattern=[[1, K]], base=0, channel_multiplier=K - 1,
                allow_small_or_imprecise_dtypes=True,
            )
            nc.gpsimd.tensor_scalar_mul(obase[:], obase[:], float(D))
            ones40 = pool.tile([B_LOC * K, 1], f32)
            nc.gpsimd.memset(ones40[:], 1.0)

            # ---- stage 1: chunked load + per-partition top-8 ----
            # all chunks on ONE ring: serial full-bandwidth transfers give the
            # earliest chunk0 (two rings split HBM bandwidth and delay it)
            xs = []
            for c in range(NCHK):
                xc = pool.tile([P, CWS[c]], f32, tag=f"x{c}")
                seg = x_dram[P * COFF[c] : P * (COFF[c] + CWS[c])]
                nc.sync.dma_start(
                    out=xc[:], in_=seg.rearrange("(p w) -> p w", w=CWS[c])
                )
                xs.append(xc)

            v32 = pool.tile([P, 8 * NCHK], f32)   # candidate values
            i32u = pool.tile([P, 8 * NCHK], u32)
            i32f = pool.tile([P, 8 * NCHK], f32)
            g32 = pool.tile([P, 8 * NCHK], f32)   # candidate flat indices
            for c in range(NCHK):
                sl = slice(8 * c, 8 * c + 8)
                nc.vector.max(out=v32[:, sl], in_=xs[c][:])
                nc.vector.max_index(
                    out=i32u[:, sl], in_max=v32[:, sl], in_values=xs[c][:]
                )
                # last chunk's index ops on Vector: no cross-engine wait on the
                # critical path into stage 2
                eng = nc.vector if c == NCHK - 1 else nc.gpsimd
                eng.tensor_copy(out=i32f[:, sl], in_=i32u[:, sl])
                eng.tensor_scalar(
                    out=g32[:, sl], in0=i32f[:, sl], scalar1=pbase[:, 0:1],
                    scalar2=float(COFF[c]), op0=ADD, op1=ADD,
                )

            # ---- stage 2: prefilter 32 -> 8 per partition ----
            vg2v = pool.tile([P, 8], f32)   # prefiltered values (MAX8 emits 8)
            vg2i = pool.tile([P, K], f32)   # flat indices of the top-5 kept
            nc.vector.max(out=vg2v[:], in_=v32[:])
            c8p = pool.tile([P, 8], u32)
            nc.vector.max_index(out=c8p[:], in_max=vg2v[:], in_values=v32[:])
            # values half of the regroup can fly while the index half computes:
            # flat order of [128,0:5] (p-major) == [8,80] (r-major) -> one DMA
            v128 = pool.tile([B_LOC, NCAND], f32)
            nc.sync.dma_start(out=v128[:], in_=vg2v[:, 0:K], single_packet=True)
            c8pf = pool.tile([P, K], f32)
            nc.vector.tensor_copy(out=c8pf[:], in_=c8p[:, 0:K])
            mp = pool.tile([P, K * 8 * NCHK], f32)
            mp3 = mp[:].rearrange("p (j c) -> p j c", j=K)
            nc.vector.tensor_tensor(
                out=mp3, in0=iota832[:].rearrange("p (j c) -> p j c", j=K),
                in1=c8pf[:].to_broadcast([P, K, 8 * NCHK]), op=EQ,
            )
            nc.vector.tensor_tensor(
                out=mp3, in0=mp3, in1=_mid_bcast(g32[:], K), op=MUL
            )
            nc.vector.tensor_reduce(out=vg2i[:], in_=mp3, axis=AXX, op=ADD)
            idx2 = pool.tile([B_LOC, NCAND], f32)
            nc.sync.dma_start(out=idx2[:], in_=vg2i[:], single_packet=True)

            # ---- stage 3: rank the 80 candidates per row ----
            t8 = pool.tile([B_LOC, 8], f32)
            nc.vector.max(out=t8[:], in_=v128[:])
            c8 = pool.tile([B_LOC, 8], u32)
            nc.vector.max_index(out=c8[:], in_max=t8[:], in_values=v128[:])
            c8f = pool.tile([B_LOC, 8], f32)
            nc.vector.tensor_copy(out=c8f[:], in_=c8[:])

            # ---- stage 4: gather the 5 winners' flat indices ----
            m5 = pool.tile([B_LOC, K * NCAND], f32)
            m53 = m5[:].rearrange("p (j c) -> p j c", j=K)
            nc.vector.tensor_tensor(
                out=m53, in0=iota5128[:].rearrange("p (j c) -> p j c", j=K),
                in1=c8f[:, 0:K].to_broadcast([B_LOC, K, NCAND]), op=EQ,
            )
            nc.vector.tensor_tensor(
                out=m53, in0=m53, in1=_mid_bcast(idx2[:], K), op=MUL
            )
            gsel = pool.tile([B_LOC, K], f32)
            nc.vector.tensor_reduce(out=gsel[:], in_=m53, axis=AXX, op=ADD)

            # ---- outputs ----
            # hard_indices = flat - r*D  (gpsimd, parallel with the off chain)
            hard_f = pool.tile([B_LOC, K], f32)
            nc.gpsimd.tensor_scalar(
                out=hard_f[:], in0=gsel[:], scalar1=rowbase[:, 0:1], scalar2=None,
                op0=SUB,
            )
            hard_t = pool.tile([B_LOC, K], i32)
            nc.gpsimd.tensor_copy(out=hard_t[:], in_=hard_f[:])
            nc.scalar.dma_start(out=hard_dram[:], in_=hard_t[:])

            # scatter offsets: flat + (4r + j)*D = (5r + j)*D + within
            off_f = pool.tile([B_LOC, K], f32)
            nc.vector.tensor_tensor(out=off_f[:], in0=gsel[:], in1=obase[:], op=ADD)
            off = pool.tile([B_LOC, K], i32)
            nc.vector.tensor_copy(out=off[:], in_=off_f[:])
            # regroup [8,5] -> [40,1]: flat element order matches, one DMA
            # (the HW DGE consumes one offset per partition)
            off40 = pool.tile([B_LOC * K, 1], i32)
            nc.sync.dma_start(out=off40[:], in_=off[:], single_packet=True)
            nc.gpsimd.indirect_dma_start(
                out=soft_dram[:].rearrange("(a b) -> a b", b=1),
                out_offset=IndirectOffsetOnAxis(ap=off40[:], axis=0),
                in_=ones40[:],
                in_offset=None,
            )

    nc.compile()
    return nc


def get_nc():
    global _NC_CACHE
    if _NC_CACHE is None:
        _NC_CACHE = build_nc()
    return _NC_CACHE


def _kernel_subprocess(similarities: np.ndarray) -> tuple[np.ndarray, np.ndarray]:
    import os
    import subprocess
    import sys
    import tempfile

    here = os.path.dirname(os.path.abspath(__file__))
    with tempfile.TemporaryDirectory() as td:
        np.save(os.path.join(td, "in.npy"), similarities)
        code = (
            "import sys, numpy as np\n"
            f"sys.path.insert(0, {here!r})\n"
            "import kernel as KM\n"
            f"sims = np.load({os.path.join(td, 'in.npy')!r})\n"
            "hard, soft = KM.kernel(sims, _no_fallback=True)\n"
            f"np.save({os.path.join(td, 'hard.npy')!r}, hard)\n"
            f"np.save({os.path.join(td, 'soft.npy')!r}, soft)\n"
        )
        subprocess.run([sys.executable, "-c", code], check=True, cwd=td)
        return (
            np.load(os.path.join(td, "hard.npy")),
            np.load(os.path.join(td, "soft.npy")),
        )


def kernel(
    similarities: np.ndarray,
    _trace: bool = False,
    _tmpdir: str | None = None,
    _no_fallback: bool = False,
) -> tuple[np.ndarray, np.ndarray]:
    similarities = np.ascontiguousarray(np.asarray(similarities, dtype=np.float32))
    assert similarities.shape == (B, D)

    nc = get_nc()

    def chunked(block):
        b2 = block.reshape(P, FREE)
        return np.concatenate(
            [b2[:, COFF[c] : COFF[c] + CWS[c]].ravel() for c in range(NCHK)]
        )

    in_maps = [
        {"sims": chunked(similarities[c * B_LOC : (c + 1) * B_LOC])}
        for c in range(N_CORES)
    ]
    try:
        res = run_bass_kernel_spmd(
            nc, in_maps, core_ids=list(range(N_CORES)), trace=_trace, tmpdir=_tmpdir
        )
    except Exception:
        if _no_fallback:
            raise
        # rare transient device faults: isolate a retry in a fresh process
        return _kernel_subprocess(similarities)
    results = res.results if hasattr(res, "results") else res
    if _trace:
        kernel.last_results = res

    hard = np.concatenate(
        [np.asarray(r["hard"], dtype=np.int32) for r in results], axis=0
    )
    soft = np.concatenate(
        [
            np.asarray(r["soft"], dtype=np.float32).reshape(B_LOC, K, D)
            for r in results
        ],
        axis=0,
    )
    return hard, soft


# revision 47
# speedup vs baseline: 1.0382x; 1.0241x over previous
"""Differentiable top-k (topk_masking) Trainium2 Bass kernel.

Problem: similarities [64, 131072] f32 ->
  hard_indices [64, 5] int32  (top-5 per row, descending, ties by lowest index)
  soft_weights [64, 5, 131072] f32

The reference's straight-through output `(onehot + p) - p` is exactly 0.0 at
non-selected positions and within 1 ulp of 1.0 at the selected position, so the
kernel emits an exact one-hot (the runtime pre-zeroes ExternalOutput buffers;
only the 5 ones per row are scattered).

Sharding: pure data parallel — 8 rows per core across 8 NeuronCores.

Per-core pipeline (local block [8, 131072] viewed as [128, 8192]; partition p
holds row p//16, in-row chunk p%16):
  1. Input is loaded in 4 uneven column-chunks (small ones first); per chunk,
     MAX8 + FIND_INDEX8 give the per-partition top-8 (values + flat indices),
     so the Vector engine pipelines behind the DMA.
  2. The 32 per-partition candidates are prefiltered back to top-5 (MAX8 over
     the 32, then a one-hot dot to gather their flat indices).
  3. Identity-order SBUF->SBUF DMAs regroup each row's 16x(5 vals + 5 idx)
     candidates into one partition; MAX8/FIND_INDEX8 rank the 80 candidates
     (duplicate-value index semantics match jax.lax.top_k tie order).
  4. The winning flat indices drive an indirect-DMA scatter of 1.0s into the
     dense output, plus the hard_indices store.
"""

import numpy as np

import concourse.bacc as bacc
import concourse.bass as bass
import concourse.mybir as mybir
import concourse.tile as tile
from concourse.bass import IndirectOffsetOnAxis
from concourse.bass_utils import run_bass_kernel_spmd

B = 64
D = 131072
K = 5
N_CORES = 8
B_LOC = B // N_CORES          # 8 rows per core
P = 128                       # SBUF partitions
FREE = B_LOC * D // P         # 8192 elems per partition
NCH = P // B_LOC              # 16 partitions (chunks) per row
NCHK = 4                      # input load chunks
CWS = [1024, 1536, 2304, 3328]  # small first chunks start Vector early
COFF = [0, 1024, 2560, 4864]    # per-partition column offsets
assert sum(CWS) == FREE and COFF[-1] + CWS[-1] == FREE
NCAND = NCH * K               # 80 candidates per row after prefilter (top-5
                              # per partition provably covers the row top-5)

f32 = mybir.dt.float32
i32 = mybir.dt.int32
u32 = mybir.dt.uint32
ADD = mybir.AluOpType.add
SUB = mybir.AluOpType.subtract
MUL = mybir.AluOpType.mult
EQ = mybir.AluOpType.is_equal
AXX = mybir.AxisListType.X

_NC_CACHE = None


def _mid_bcast(ap, n):
    """[p, m] -> [p, n, m] with a step-0 middle dim."""
    dims = [list(d) for d in ap.ap]
    return bass.AP(ap.tensor, ap.offset, [dims[0], [0, n]] + dims[1:])


def build_nc():
    nc = bacc.Bacc("TRN2", target_bir_lowering=False, debug=False)
    # chunk-major input layout: chunk c is a contiguous [128, CWS[c]] block
    x_dram = nc.dram_tensor("sims", [P * FREE], f32, kind="ExternalInput")
    soft_dram = nc.dram_tensor("soft", [B_LOC * K * D], f32, kind="ExternalOutput")
    hard_dram = nc.dram_tensor("hard", [B_LOC, K], i32, kind="ExternalOutput")

    with tile.TileContext(nc) as tc:
        with tc.tile_pool(name="sb", bufs=1) as pool:
            # ---- constants, no deps (gpsimd; overlap the input DMA) ----
            pbase = pool.tile([P, 1], f32)  # p * FREE
            nc.gpsimd.iota(
                pbase[:], pattern=[[0, 1]], base=0, channel_multiplier=FREE,
                allow_small_or_imprecise_dtypes=True,
            )
            iota832 = pool.tile([P, K * 8 * NCHK], f32)  # c in 0..31, repeated 5x
            nc.gpsimd.iota(
                iota832[:], pattern=[[0, K], [1, 8 * NCHK]], base=0,
                channel_multiplier=0, allow_small_or_imprecise_dtypes=True,
            )
            iota5128 = pool.tile([B_LOC, K * NCAND], f32)  # c in 0..127, rep 5x
            nc.gpsimd.iota(
                iota5128[:], pattern=[[0, K], [1, NCAND]], base=0,
                channel_multiplier=0, allow_small_or_imprecise_dtypes=True,
            )
            rowbase = pool.tile([B_LOC, 1], f32)  # r * D
            nc.gpsimd.iota(
                rowbase[:], pattern=[[0, 1]], base=0, channel_multiplier=1,
                allow_small_or_imprecise_dtypes=True,
            )
            nc.gpsimd.tensor_scalar_mul(rowbase[:], rowbase[:], float(D))
            obase = pool.tile([B_LOC, K], f32)  # (4r + j) * D
            nc.gpsimd.iota(
                obase[:], pattern=[[1, K]], base=0, channel_multiplier=K - 1,
                allow_small_or_imprecise_dtypes=True,
            )
            nc.gpsimd.tensor_scalar_mul(obase[:], obase[:], float(D))
            ones40 = pool.tile([B_LOC * K, 1], f32)
            nc.gpsimd.memset(ones40[:], 1.0)

            # ---- stage 1: chunked load + per-partition top-8 ----
            # all chunks on ONE ring: serial full-bandwidth transfers give the
            # earliest chunk0 (two rings split HBM bandwidth and delay it)
            xs = []
            for c in range(NCHK):
                xc = pool.tile([P, CWS[c]], f32, tag=f"x{c}")
                seg = x_dram[P * COFF[c] : P * (COFF[c] + CWS[c])]
                nc.sync.dma_start(
                    out=xc[:], in_=seg.rearrange("(p w) -> p w", w=CWS[c])
                )
                xs.append(xc)

            v32 = pool.tile([P, 8 * NCHK], f32)   # candidate values
            i32u = pool.tile([P, 8 * NCHK], u32)
            i32f = pool.tile([P, 8 * NCHK], f32)
            g32 = pool.tile([P, 8 * NCHK], f32)   # candidate flat indices
            for c in range(NCHK):
                sl = slice(8 * c, 8 * c + 8)
                nc.vector.max(out=v32[:, sl], in_=xs[c][:])
                nc.vector.max_index(
                    out=i32u[:, sl], in_max=v32[:, sl], in_values=xs[c][:]
                )
                # last chunk's index ops on Vector: no cross-engine wait on the
                # critical path into stage 2
                eng = nc.vector if c == NCHK - 1 else nc.gpsimd
                eng.tensor_copy(out=i32f[:, sl], in_=i32u[:, sl])
                eng.tensor_scalar(
                    out=g32[:, sl], in0=i32f[:, sl], scalar1=pbase[:, 0:1],
                    scalar2=float(COFF[c]), op0=ADD, op1=ADD,
                )

            # ---- stage 2: prefilter 32 -> 8 per partition ----
            vg2v = pool.tile([P, 8], f32)   # prefiltered values (MAX8 emits 8)
            vg2i = pool.tile([P, K], f32)   # flat indices of the top-5 kept
            nc.vector.max(out=vg2v[:], in_=v32[:])
            c8p = pool.tile([P, 8], u32)
            nc.vector.max_index(out=c8p[:], in_max=vg2v[:], in_values=v32[:])
            # values half of the regroup can fly while the index half computes:
            # flat order of [128,0:5] (p-major) == [8,80] (r-major) -> one DMA
            v128 = pool.tile([B_LOC, NCAND], f32)
            nc.sync.dma_start(out=v128[:], in_=vg2v[:, 0:K], single_packet=True)
            c8pf = pool.tile([P, K], f32)
            nc.vector.tensor_copy(out=c8pf[:], in_=c8p[:, 0:K])
            mp = pool.tile([P, K * 8 * NCHK], f32)
            mp3 = mp[:].rearrange("p (j c) -> p j c", j=K)
            nc.vector.tensor_tensor(
                out=mp3, in0=iota832[:].rearrange("p (j c) -> p j c", j=K),
                in1=c8pf[:].to_broadcast([P, K, 8 * NCHK]), op=EQ,
            )
            nc.vector.tensor_tensor(
                out=mp3, in0=mp3, in1=_mid_bcast(g32[:], K), op=MUL
            )
            nc.vector.tensor_reduce(out=vg2i[:], in_=mp3, axis=AXX, op=ADD)
            idx2 = pool.tile([B_LOC, NCAND], f32)
            nc.sync.dma_start(out=idx2[:], in_=vg2i[:], single_packet=True)

            # ---- stage 3: rank the 80 candidates per row ----
            t8 = pool.tile([B_LOC, 8], f32)
            nc.vector.max(out=t8[:], in_=v128[:])
            c8 = pool.tile([B_LOC, 8], u32)
            nc.vector.max_index(out=c8[:], in_max=t8[:], in_values=v128[:])
            c8f = pool.tile([B_LOC, 8], f32)
            nc.vector.tensor_copy(out=c8f[:], in_=c8[:])

            # ---- stage 4: gather the 5 winners' flat indices ----
            m5 = pool.tile([B_LOC, K * NCAND], f32)
            m53 = m5[:].rearrange("p (j c) -> p j c", j=K)
            nc.vector.tensor_tensor(
                out=m53, in0=iota5128[:].rearrange("p (j c) -> p j c", j=K),
                in1=c8f[:, 0:K].to_broadcast([B_LOC, K, NCAND]), op=EQ,
            )
            nc.vector.tensor_tensor(
                out=m53, in0=m53, in1=_mid_bcast(idx2[:], K), op=MUL
            )
            gsel = pool.tile([B_LOC, K], f32)
            nc.vector.tensor_reduce(out=gsel[:], in_=m53, axis=AXX, op=ADD)

            # ---- outputs ----
            # hard_indices = flat - r*D  (gpsimd, parallel with the off chain)
            hard_f = pool.tile([B_LOC, K], f32)
            nc.gpsimd.tensor_scalar(
                out=hard_f[:], in0=gsel[:], scalar1=rowbase[:, 0:1], scalar2=None,
                op0=SUB,
            )
            hard_t = pool.tile([B_LOC, K], i32)
            nc.gpsimd.tensor_copy(out=hard_t[:], in_=hard_f[:])
            nc.scalar.dma_start(out=hard_dram[:], in_=hard_t[:])

            # scatter offsets: flat + (4r + j)*D = (5r + j)*D + within
            off_f = pool.tile([B_LOC, K], f32)
            nc.vector.tensor_tensor(out=off_f[:], in0=gsel[:], in1=obase[:], op=ADD)
            off = pool.tile([B_LOC, K], i32)
            nc.vector.tensor_copy(out=off[:], in_=off_f[:])
            # regroup [8,5] -> [40,1]: flat element order matches, one DMA
            # (the HW DGE consumes one offset per partition)
            off40 = pool.tile([B_LOC * K, 1], i32)
            nc.sync.dma_start(out=off40[:], in_=off[:], single_packet=True)
            nc.gpsimd.indirect_dma_start(
                out=soft_dram[:].rearrange("(a b) -> a b", b=1),
                out_offset=IndirectOffsetOnAxis(ap=off40[:], axis=0),
                in_=ones40[:],
                in_offset=None,
            )

    nc.compile()
    return nc


def get_nc():
    global _NC_CACHE
    if _NC_CACHE is None:
        _NC_CACHE = build_nc()
    return _NC_CACHE


def _kernel_subprocess(similarities: np.ndarray) -> tuple[np.ndarray, np.ndarray]:
    import os
    import subprocess
    import sys
    import tempfile

    here = os.path.dirname(os.path.abspath(__file__))
    with tempfile.TemporaryDirectory() as td:
        np.save(os.path.join(td, "in.npy"), similarities)
        code = (
            "import sys, numpy as np\n"
            f"sys.path.insert(0, {here!r})\n"
            "import kernel as KM\n"
            f"sims = np.load({os.path.join(td, 'in.npy')!r})\n"
            "hard, soft = KM.kernel(sims, _no_fallback=True)\n"
            f"np.save({os.path.join(td, 'hard.npy')!r}, hard)\n"
            f"np.save({os.path.join(td, 'soft.npy')!r}, soft)\n"
        )
        subprocess.run([sys.executable, "-c", code], check=True, cwd=td)
        return (
            np.load(os.path.join(td, "hard.npy")),
            np.load(os.path.join(td, "soft.npy")),
        )


def kernel(
    similarities: np.ndarray,
    _trace: bool = False,
    _tmpdir: str | None = None,
    _no_fallback: bool = False,
) -> tuple[np.ndarray, np.ndarray]:
    similarities = np.ascontiguousarray(np.asarray(similarities, dtype=np.float32))
    assert similarities.shape == (B, D)

    nc = get_nc()

    def chunked(block):
        b2 = block.reshape(P, FREE)
        return np.concatenate(
            [b2[:, COFF[c] : COFF[c] + CWS[c]].ravel() for c in range(NCHK)]
        )

    in_maps = [
        {"sims": chunked(similarities[c * B_LOC : (c + 1) * B_LOC])}
        for c in range(N_CORES)
    ]
    try:
        res = run_bass_kernel_spmd(
            nc, in_maps, core_ids=list(range(N_CORES)), trace=_trace, tmpdir=_tmpdir
        )
    except Exception:
        if _no_fallback:
            raise
        # rare transient device faults: isolate a retry in a fresh process
        return _kernel_subprocess(similarities)
    results = res.results if hasattr(res, "results") else res
    if _trace:
        kernel.last_results = res

    hard = np.concatenate(
        [np.asarray(r["hard"], dtype=np.int32) for r in results], axis=0
    )
    soft = np.concatenate(
        [
            np.asarray(r["soft"], dtype=np.float32).reshape(B_LOC, K, D)
            for r in results
        ],
        axis=0,
    )
    return hard, soft
